# revision 1
# baseline (speedup 1.0000x reference)
"""Two-layer GraphSAGE (mean aggregation) on 8 Trainium2 NeuronCores.

Strategy (matches the dst-partitioning hint):
- Nodes are partitioned by destination across 8 cores (12500 nodes each,
  padded to 12544 = 98*128). Each core owns the edges whose dst lands in
  its slice, pre-sorted/bucketed by (core, dst-tile) on the host.
- x is replicated to every core in a padded layout so src indices are
  identical for both layers. Layer-1 aggregation gathers x[src] rows with
  large indirect DMAs, reduces them per 128-dst tile with indicator
  matmuls on the TensorEngine (indicator built on DVE from host-provided
  dst offsets), and applies mean + the two dense 128x128 matmuls.
- Between layers, each core's h slice is AllGathered so layer 2 can
  gather h[src] for remote sources. Layer-2 self term reads the local
  (pre-AllGather) slice.

kernel(**inputs) -> np.ndarray takes the FULL inputs and returns the FULL
[100000, 128] output; all sharding/unsharding happens inside.
"""

import math
import os

import numpy as np

P = 128
NCORES = 8


def _prep_edges(edge_index: np.ndarray, n_nodes: int, npc: int, tpc: int):
    """Bucket edges by (owner core, dst tile); pad each tile to whole
    128-edge chunks (uniform chunk count across cores per tile so the SPMD
    program is identical on every core).

    Returns (ch, coloff, ncols, esrc, edst):
      ch[t]    : number of 128-edge chunks for dst tile t (max over cores)
      coloff[t]: starting column of tile t in the packed arrays
      esrc     : [8, 128, ncols] int32, padded-global src ids (pad = 0)
      edst     : [8, 128, ncols] float32, dst offset within tile (pad = -1)
    """
    npc_pad = tpc * P
    src = edge_index[0].astype(np.int64)
    dst = edge_index[1].astype(np.int64)
    srcpad = ((src // npc) * npc_pad + (src % npc)).astype(np.int64)
    core = dst // npc
    loc = dst % npc
    tl = loc // P
    off = loc % P

    key = core * tpc + tl
    counts = np.bincount(key, minlength=NCORES * tpc).reshape(NCORES, tpc)
    ch = np.maximum(1, -(-counts.max(axis=0) // P)).astype(np.int64)
    coloff = np.zeros(tpc + 1, np.int64)
    np.cumsum(ch, out=coloff[1:])
    ncols = int(coloff[-1])

    esrc = np.zeros((NCORES, ncols * P), np.int32)
    edst = np.full((NCORES, ncols * P), -1.0, np.float32)

    order = np.argsort(key, kind="stable")
    sk = key[order]
    first = np.r_[True, sk[1:] != sk[:-1]]
    idx_of_first = np.where(first)[0]
    grp_id = np.cumsum(first) - 1
    rank = np.arange(len(sk)) - idx_of_first[grp_id]
    slot = coloff[tl[order]] * P + rank
    esrc[core[order], slot] = srcpad[order].astype(np.int32)
    edst[core[order], slot] = off[order].astype(np.float32)

    esrc = np.ascontiguousarray(esrc.reshape(NCORES, ncols, P).transpose(0, 2, 1))
    edst = np.ascontiguousarray(edst.reshape(NCORES, ncols, P).transpose(0, 2, 1))

    # per-node 1/max(indegree,1), laid out [core][partition, tile]
    cnt = np.bincount(dst, minlength=n_nodes).astype(np.float32)
    recip = np.zeros((NCORES, npc_pad), np.float32)
    for c in range(NCORES):
        recip[c, :npc] = 1.0 / np.maximum(cnt[c * npc : (c + 1) * npc], 1.0)
    recip = np.ascontiguousarray(recip.reshape(NCORES, tpc, P).transpose(0, 2, 1))
    return ch, coloff, ncols, esrc, edst, recip


def _gather_groups(ch, coloff, tpc, gmax):
    """Group consecutive dst tiles so each group's gather is one indirect
    DMA of at most gmax columns (gmax*128 rows)."""
    groups = []
    t = 0
    while t < tpc:
        t0 = t
        cols = 0
        while t < tpc and cols + ch[t] <= gmax:
            cols += ch[t]
            t += 1
        groups.append((t0, t, int(coloff[t0]), int(coloff[t])))
    return groups


def _build_program(tpc, ncols, ch, coloff, groups, n_all_pad):
    from concourse import bacc, bass, mybir, tile

    npc_pad = tpc * P
    f32 = mybir.dt.float32
    i32 = mybir.dt.int32

    nc = bacc.Bacc(
        "TRN2", target_bir_lowering=False, debug=False, num_devices=NCORES
    )

    xg = nc.declare_dram_parameter("xg", [n_all_pad, P], f32, isOutput=False)
    xown = nc.declare_dram_parameter("xown", [npc_pad, P], f32, isOutput=False)
    esrc_d = nc.declare_dram_parameter("esrc", [P, ncols], i32, isOutput=False)
    edst_d = nc.declare_dram_parameter("edst", [P, ncols], f32, isOutput=False)
    wl1_d = nc.declare_dram_parameter("wl1", [P, P], f32, isOutput=False)
    wr1_d = nc.declare_dram_parameter("wr1", [P, P], f32, isOutput=False)
    wl2_d = nc.declare_dram_parameter("wl2", [P, P], f32, isOutput=False)
    wr2_d = nc.declare_dram_parameter("wr2", [P, P], f32, isOutput=False)
    bias1_d = nc.declare_dram_parameter("bias1", [P, P], f32, isOutput=False)
    bias2_d = nc.declare_dram_parameter("bias2", [P, P], f32, isOutput=False)
    iota_d = nc.declare_dram_parameter("iota", [P, P], f32, isOutput=False)
    ident_d = nc.declare_dram_parameter("ident", [P, P], f32, isOutput=False)
    recip_d = nc.declare_dram_parameter("recip", [P, tpc], f32, isOutput=False)
    out_d = nc.declare_dram_parameter("out", [npc_pad, P], f32, isOutput=True)

    gmax = max(g[3] - g[2] for g in groups)

    with tile.TileContext(nc) as tc:
        with (
            tc.tile_pool(name="const", bufs=1) as cpool,
            tc.tile_pool(name="meta", bufs=1) as mpool,
            tc.tile_pool(name="gath", bufs=2) as gpool,
            tc.tile_pool(name="work", bufs=3) as wpool,
            tc.tile_pool(name="psacc", bufs=2, space="PSUM") as ps_acc,
            tc.tile_pool(name="psself", bufs=2, space="PSUM") as ps_self,
            tc.tile_pool(name="psh", bufs=2, space="PSUM") as ps_h,
            tc.tile_pool(name="dram", bufs=1, space="DRAM") as dpool,
        ):
            def load_const(dram_ap, shape, dtype=f32, name=None):
                t = cpool.tile(shape, dtype, name=name)
                nc.sync.dma_start(out=t[:], in_=dram_ap)
                return t

            wl1 = load_const(wl1_d[:], [P, P], name="wl1")
            wr1 = load_const(wr1_d[:], [P, P], name="wr1")
            wl2 = load_const(wl2_d[:], [P, P], name="wl2")
            wr2 = load_const(wr2_d[:], [P, P], name="wr2")
            bias1 = load_const(bias1_d[:], [P, P], name="bias1")
            bias2 = load_const(bias2_d[:], [P, P], name="bias2")
            iota = load_const(iota_d[:], [P, P], name="iota")
            ident = load_const(ident_d[:], [P, P], name="ident")
            recip = load_const(recip_d[:], [P, tpc], name="recip")
            esrc = mpool.tile([P, ncols], i32, name="esrc")
            nc.sync.dma_start(out=esrc[:], in_=esrc_d[:])
            edst = mpool.tile([P, ncols], f32, name="edst")
            nc.sync.dma_start(out=edst[:], in_=edst_d[:])

            h_bounce = dpool.tile([npc_pad, P], f32, name="h_bounce")
            h_full = dpool.tile(
                [n_all_pad, P], f32, name="h_full", addr_space="Shared"
            )

            def layer(src_table, self_src, dst_dram, wl, wr, bias, relu):
                for (t0, t1, c0, c1) in groups:
                    g_sb = gpool.tile([P, gmax * P], f32, tag="gath")
                    # HW indirect DMA consumes ONE offset per partition, so
                    # gather 128 rows per instruction (one per chunk column).
                    for cc in range(c0, c1):
                        nc.gpsimd.indirect_dma_start(
                            out=g_sb[:, (cc - c0) * P : (cc - c0 + 1) * P],
                            out_offset=None,
                            in_=src_table[:],
                            in_offset=bass.IndirectOffsetOnAxis(
                                ap=esrc[:, cc : cc + 1], axis=0
                            ),
                        )
                    for t in range(t0, t1):
                        cht = int(ch[t])
                        tc0 = int(coloff[t]) - c0
                        ind = wpool.tile([P, cht, P], f32, tag="ind")
                        nc.vector.tensor_tensor(
                            out=ind[:],
                            in0=edst[:, coloff[t] : coloff[t] + cht, None]
                            .to_broadcast([P, cht, P]),
                            in1=iota[:, None, :].to_broadcast([P, cht, P]),
                            op=mybir.AluOpType.is_equal,
                        )
                        acc = ps_acc.tile([P, P], f32, tag="acc")
                        for k in range(cht):
                            nc.tensor.matmul(
                                out=acc[:],
                                lhsT=g_sb[:, (tc0 + k) * P : (tc0 + k + 1) * P],
                                rhs=ind[:, k, :],
                                start=(k == 0),
                                stop=(k == cht - 1),
                            )
                        # self term: x_own[t] transposed via PE
                        xo = wpool.tile([P, P], f32, tag="xo")
                        nc.sync.dma_start(
                            out=xo[:], in_=self_src[t * P : (t + 1) * P, :]
                        )
                        selfT_ps = ps_self.tile([P, P], f32, tag="selfT")
                        nc.tensor.transpose(
                            out=selfT_ps[:], in_=xo[:], identity=ident[:]
                        )
                        selfT = wpool.tile([P, P], f32, tag="selfT_sb")
                        nc.vector.tensor_copy(out=selfT[:], in_=selfT_ps[:])
                        aggT = wpool.tile([P, P], f32, tag="aggT_sb")
                        nc.vector.tensor_copy(out=aggT[:], in_=acc[:])
                        h1 = ps_h.tile([P, P], f32, tag="h1")
                        nc.tensor.matmul(
                            out=h1[:], lhsT=aggT[:], rhs=wl[:],
                            start=True, stop=True,
                        )
                        h2 = ps_h.tile([P, P], f32, tag="h2")
                        nc.tensor.matmul(
                            out=h2[:], lhsT=selfT[:], rhs=wr[:],
                            start=True, stop=True,
                        )
                        hsb = wpool.tile([P, P], f32, tag="hsb")
                        nc.vector.tensor_scalar_mul(
                            out=hsb[:], in0=h1[:], scalar1=recip[:, t : t + 1]
                        )
                        nc.vector.tensor_add(out=hsb[:], in0=hsb[:], in1=h2[:])
                        nc.vector.tensor_add(out=hsb[:], in0=hsb[:], in1=bias[:])
                        if relu:
                            nc.scalar.activation(
                                out=hsb[:], in_=hsb[:],
                                func=mybir.ActivationFunctionType.Relu,
                            )
                        nc.sync.dma_start(
                            out=dst_dram[t * P : (t + 1) * P, :], in_=hsb[:]
                        )

            layer(xg, xown, h_bounce, wl1, wr1, bias1, relu=True)
            nc.gpsimd.collective_compute(
                "AllGather",
                mybir.AluOpType.bypass,
                replica_groups=[list(range(NCORES))],
                ins=[h_bounce[:]],
                outs=[h_full[:]],
            )
            layer(h_full, h_bounce, out_d, wl2, wr2, bias2, relu=False)

    return nc


def run(x, edge_index, W_l1, b_l1, W_r1, W_l2, b_l2, W_r2, trace=False):
    n_nodes = x.shape[0]
    assert n_nodes % NCORES == 0
    npc = n_nodes // NCORES
    tpc = -(-npc // P)
    npc_pad = tpc * P
    n_all_pad = NCORES * npc_pad
    gmax = int(os.environ.get("SAGE_GMAX", "24"))

    ch, coloff, ncols, esrc, edst, recip = _prep_edges(
        edge_index, n_nodes, npc, tpc
    )
    groups = _gather_groups(ch, coloff, tpc, gmax)

    x = np.asarray(x, np.float32)
    x_pad = np.zeros((n_all_pad, P), np.float32)
    for c in range(NCORES):
        x_pad[c * npc_pad : c * npc_pad + npc] = x[c * npc : (c + 1) * npc]

    common = {
        "xg": x_pad,
        "wl1": np.asarray(W_l1, np.float32),
        "wr1": np.asarray(W_r1, np.float32),
        "wl2": np.asarray(W_l2, np.float32),
        "wr2": np.asarray(W_r2, np.float32),
        "bias1": np.ascontiguousarray(
            np.broadcast_to(np.asarray(b_l1, np.float32), (P, P))
        ),
        "bias2": np.ascontiguousarray(
            np.broadcast_to(np.asarray(b_l2, np.float32), (P, P))
        ),
        "iota": np.ascontiguousarray(
            np.broadcast_to(np.arange(P, dtype=np.float32), (P, P))
        ),
        "ident": np.eye(P, dtype=np.float32),
    }
    in_maps = []
    for c in range(NCORES):
        m = dict(common)
        m["xown"] = np.ascontiguousarray(x_pad[c * npc_pad : (c + 1) * npc_pad])
        m["esrc"] = esrc[c]
        m["edst"] = edst[c]
        m["recip"] = recip[c]
        in_maps.append(m)

    nc = _build_program(tpc, ncols, ch, coloff, groups, n_all_pad)
    nc.finalize()

    from concourse.bass_utils import run_bass_kernel_spmd

    res = run_bass_kernel_spmd(
        nc, in_maps, list(range(NCORES)), trace=trace,
    )
    out = np.empty((n_nodes, P), np.float32)
    for c in range(NCORES):
        out[c * npc : (c + 1) * npc] = res.results[c]["out"][:npc]
    return out, res


def kernel(x, edge_index, W_l1, b_l1, W_r1, W_l2, b_l2, W_r2):
    out, _ = run(x, edge_index, W_l1, b_l1, W_r1, W_l2, b_l2, W_r2)
    return out



# revision 7
# speedup vs baseline: 1.0305x; 1.0305x over previous
"""Two-layer GraphSAGE (mean aggregation) on 8 Trainium2 NeuronCores.

Strategy (dst-partitioning per the hint), v2:
- Nodes partitioned by destination across 8 cores (12500 each, padded to
  12544 = 98*128 rows). Each core owns edges whose dst is in its slice,
  bucketed on host by (dst tile, src window) where the 4 windows of
  <=32768 rows make gather indices fit int16.
- Aggregation per 128-dst tile: gather x[src] rows in bf16 with BATCHED
  SWDGE dma_gather instructions (thousands of descriptors each, replacing
  per-chunk indirect DMAs), reduce with indicator matmuls on the PE
  (indicator = is_equal(iota, dstoff) * (1/deg) built in one fused DVE
  tensor_scalar, so the mean is folded into the accumulation).
- Everything downstream is kept in transposed [feat, node] layout:
  hT = W_l^T @ aggT + W_r^T @ xT, so bias+relu+PSUM-drain is a single
  Activation-engine op (bias is per-partition), and the layer-2 self term
  reuses layer-1's resident hT tile directly. Row-layout copies of h (for
  the gather table / final output) are made with PE transposes and written
  with one batched DMA per tile-group.
- Between layers the bf16 h slices are AllGathered (2 chunks, the first
  overlapped with the tail of layer-1 compute).

kernel(**inputs) -> np.ndarray takes FULL inputs, returns FULL output.
"""

import os

import numpy as np

P = 128
NCORES = 8
NPC = 12500            # nodes per core
TPC = 98               # 128-node tiles per core
NPC_PAD = TPC * P      # 12544
NALL = NCORES * NPC_PAD  # 100352
NWIN = 4
WIN = 32768            # window size so gather indices fit int16
WLIM = [min((w + 1) * WIN, NALL) for w in range(NWIN)]
WBASE = [w * WIN for w in range(NWIN)]
TPG = 14               # tiles per group -> 7 groups
DENSE_B = 4            # tiles per dense-matmul batch (512 moving free dim)


def _prep_edges(edge_index: np.ndarray, n_nodes: int):
    """Bucket edges by (owner core, dst tile, src window).

    Column layout (shared by all cores; chunk counts are max over cores so
    the SPMD program is uniform): group-major, then window, then tile.
    Returns the schedule plus per-core device arrays.
    """
    src = edge_index[0].astype(np.int64)
    dst = edge_index[1].astype(np.int64)
    srcpad = (src // NPC) * NPC_PAD + (src % NPC)
    rng = srcpad >> 15  # window index, 0..3
    core = dst // NPC
    loc = dst % NPC
    tl = loc // P
    off = loc % P

    key = (core * TPC + tl) * NWIN + rng
    cnt = np.bincount(key, minlength=NCORES * TPC * NWIN).reshape(
        NCORES, TPC, NWIN
    )
    ch = -(-cnt.max(axis=0) // P)  # [TPC, NWIN], 0 allowed
    assert ch.sum(axis=1).min() >= 1

    colof = np.zeros((TPC, NWIN), np.int64)
    groups = []  # (t0, t1, gc0, gc1, spans[r] = (c0, c1))
    c = 0
    for g0 in range(0, TPC, TPG):
        t0, t1 = g0, min(g0 + TPG, TPC)
        gc0 = c
        spans = []
        for r in range(NWIN):
            rc0 = c
            for t in range(t0, t1):
                colof[t, r] = c
                c += ch[t, r]
            spans.append((rc0, c))
        groups.append((t0, t1, gc0, c, spans))
    ncols = int(c)
    S = ncols * P

    # rank of each edge within its (core, tile, window) bucket
    order = np.argsort(key, kind="stable")
    sk = key[order]
    first = np.r_[True, sk[1:] != sk[:-1]]
    idx_of_first = np.where(first)[0]
    grp_id = np.cumsum(first) - 1
    rank = np.arange(len(sk)) - idx_of_first[grp_id]
    slot = colof[tl[order], rng[order]] * P + rank

    deg = np.bincount(dst, minlength=n_nodes).astype(np.float64)
    erec_e = 1.0 / np.maximum(deg, 1.0)

    idxflat = np.zeros((NCORES, S), np.int16)
    edstflat = np.full((NCORES, S), -1.0, np.float32)
    erecflat = np.zeros((NCORES, S), np.float32)
    idxval = (srcpad - np.asarray(WBASE, np.int64)[rng]).astype(np.int16)
    co = core[order]
    idxflat[co, slot] = idxval[order]
    edstflat[co, slot] = off[order].astype(np.float32)
    erecflat[co, slot] = erec_e[dst][order].astype(np.float32)

    idx16 = np.ascontiguousarray(
        np.tile(idxflat.reshape(NCORES, S // 16, 16).transpose(0, 2, 1), (1, 8, 1))
    )
    edst = np.ascontiguousarray(edstflat.reshape(NCORES, ncols, P).transpose(0, 2, 1))
    erec = np.ascontiguousarray(erecflat.reshape(NCORES, ncols, P).transpose(0, 2, 1))
    return ch, colof, ncols, groups, idx16, edst, erec


def _build_program(ncols, groups, tile_cols, gcmax, gmaxc):
    from concourse import bacc, library_config, mybir, tile

    f32 = mybir.dt.float32
    bf16 = mybir.dt.bfloat16
    i16 = mybir.dt.int16

    nc = bacc.Bacc(
        "TRN2",
        target_bir_lowering=False,
        debug=False,
        num_devices=NCORES,
        num_swdge_queues=4,
    )

    xg_d = nc.declare_dram_parameter("xg", [NALL, P], bf16, isOutput=False)
    xownT_d = nc.declare_dram_parameter("xownT", [P, NPC_PAD], bf16, isOutput=False)
    idx_d = nc.declare_dram_parameter("idx16", [P, (ncols * P) // 16], i16,
                                      isOutput=False)
    edst_d = nc.declare_dram_parameter("edst", [P, ncols], f32, isOutput=False)
    erec_d = nc.declare_dram_parameter("erec", [P, ncols], f32, isOutput=False)
    wl1_d = nc.declare_dram_parameter("wl1", [P, P], bf16, isOutput=False)
    wr1_d = nc.declare_dram_parameter("wr1", [P, P], bf16, isOutput=False)
    wl2_d = nc.declare_dram_parameter("wl2", [P, P], bf16, isOutput=False)
    wr2_d = nc.declare_dram_parameter("wr2", [P, P], bf16, isOutput=False)
    bias1_d = nc.declare_dram_parameter("bias1", [P, 1], f32, isOutput=False)
    bias2_d = nc.declare_dram_parameter("bias2", [P, 1], f32, isOutput=False)
    iota_d = nc.declare_dram_parameter("iota", [P, P], bf16, isOutput=False)
    ident_d = nc.declare_dram_parameter("ident", [P, P], bf16, isOutput=False)
    out_d = nc.declare_dram_parameter("out", [NPC_PAD, P], f32, isOutput=True)

    is_eq = mybir.AluOpType.is_equal
    mult = mybir.AluOpType.mult

    with tile.TileContext(nc) as tc:
        with (
            tc.tile_pool(name="const", bufs=1) as cpool,
            tc.tile_pool(name="gath", bufs=2) as gpool,
            tc.tile_pool(name="agg", bufs=2) as apool,
            tc.tile_pool(name="ind", bufs=3) as wpool,
            tc.tile_pool(name="row", bufs=2) as rpool,
            tc.tile_pool(name="psacc", bufs=2, space="PSUM") as ps_a,
            tc.tile_pool(name="psh", bufs=2, space="PSUM") as ps_h,
            tc.tile_pool(name="pst", bufs=2, space="PSUM") as ps_t,
            tc.tile_pool(name="dram", bufs=1, space="DRAM") as dpool,
        ):
            # InstDMAGatherAnt lives in the mlp Q7 ucode library
            nc.gpsimd.load_library(library_config.mlp)

            def load_const(dram_ap, shape, dtype, name):
                t = cpool.tile(shape, dtype, name=name)
                nc.sync.dma_start(out=t[:], in_=dram_ap)
                return t

            wl1 = load_const(wl1_d[:], [P, P], bf16, "wl1")
            wr1 = load_const(wr1_d[:], [P, P], bf16, "wr1")
            wl2 = load_const(wl2_d[:], [P, P], bf16, "wl2")
            wr2 = load_const(wr2_d[:], [P, P], bf16, "wr2")
            bias1 = load_const(bias1_d[:], [P, 1], f32, "bias1")
            bias2 = load_const(bias2_d[:], [P, 1], f32, "bias2")
            iota = load_const(iota_d[:], [P, P], bf16, "iota")
            ident = load_const(ident_d[:], [P, P], bf16, "ident")
            xownT = load_const(xownT_d[:], [P, NPC_PAD], bf16, "xownT")
            idx16 = load_const(idx_d[:], [P, (ncols * P) // 16], i16, "idx16")
            edst = load_const(edst_d[:], [P, ncols], f32, "edst")
            erec = load_const(erec_d[:], [P, ncols], f32, "erec")

            hT = cpool.tile([P, NPC_PAD], bf16, name="hT")

            h_bounce = dpool.tile([NPC_PAD, P], bf16, name="h_bounce")
            h_full3 = dpool.tile(
                [NCORES, NPC_PAD, P], bf16, name="h_full", addr_space="Shared"
            )
            h_full2 = h_full3[:].rearrange("c n d -> (c n) d")

            def layer(src2d, selfT, wl, wr, bias, relu, hTdst_fn, rowdst,
                      rowdtype):
                for gi, (t0, t1, gc0, gc1, spans) in enumerate(groups):
                    nt = t1 - t0
                    gbuf = gpool.tile([P, gcmax, P], bf16, tag="g")
                    for r, (c0, c1) in enumerate(spans):
                        for s0 in range(c0, c1, gmaxc):
                            s1 = min(s0 + gmaxc, c1)
                            n_idx = (s1 - s0) * P
                            nc.gpsimd.dma_gather(
                                gbuf[:, s0 - gc0 : s1 - gc0, :],
                                src2d[WBASE[r] : WLIM[r], :],
                                idx16[:, s0 * 8 : s1 * 8],
                                n_idx,
                                n_idx,
                                P,
                                queue_num=r,
                            )
                    agg = apool.tile([P, nt, P], bf16, tag="agg")
                    for ti, t in enumerate(range(t0, t1)):
                        cols = tile_cols[t]
                        acc = ps_a.tile([P, P], mybir.dt.float32, tag="acc")
                        last = len(cols) - 1
                        for ci, col in enumerate(cols):
                            ind = wpool.tile([P, P], bf16, tag="ind")
                            nc.vector.tensor_scalar(
                                out=ind[:],
                                in0=iota[:],
                                scalar1=edst[:, col : col + 1],
                                scalar2=erec[:, col : col + 1],
                                op0=is_eq,
                                op1=mult,
                            )
                            nc.tensor.matmul(
                                out=acc[:],
                                lhsT=gbuf[:, col - gc0, :],
                                rhs=ind[:],
                                start=(ci == 0),
                                stop=(ci == last),
                            )
                        nc.scalar.copy(out=agg[:, ti, :], in_=acc[:])

                    hTdst = hTdst_fn(t0, t1)
                    for b0 in range(t0, t1, DENSE_B):
                        b1 = min(b0 + DENSE_B, t1)
                        w = (b1 - b0) * P
                        hps = ps_h.tile([P, DENSE_B * P], mybir.dt.float32,
                                        tag="h")
                        nc.tensor.matmul(
                            out=hps[:, :w], lhsT=wl[:],
                            rhs=agg[:, b0 - t0 : b1 - t0, :],
                            start=True, stop=False,
                        )
                        nc.tensor.matmul(
                            out=hps[:, :w], lhsT=wr[:],
                            rhs=selfT[:, b0 * P : b1 * P],
                            start=False, stop=True,
                        )
                        nc.scalar.activation(
                            out=hTdst[:, (b0 - t0) * P : (b1 - t0) * P],
                            in_=hps[:, :w],
                            func=(
                                mybir.ActivationFunctionType.Relu
                                if relu
                                else mybir.ActivationFunctionType.Identity
                            ),
                            bias=bias[:, 0:1],
                        )

                    rowbuf = rpool.tile([P, nt, P], rowdtype, tag="row")
                    for ti in range(nt):
                        tps = ps_t.tile([P, P], bf16, tag="tp")
                        nc.tensor.transpose(
                            out=tps[:],
                            in_=hTdst[:, ti * P : (ti + 1) * P],
                            identity=ident[:],
                        )
                        nc.vector.tensor_copy(out=rowbuf[:, ti, :], in_=tps[:])
                    nc.sync.dma_start(
                        out=rowdst[t0 * P : t1 * P, :].rearrange(
                            "(t p) q -> p t q", p=P
                        ),
                        in_=rowbuf[:],
                    )

            def hT_resident(t0, t1):
                return hT[:, t0 * P : t1 * P]

            hT2_tiles = {}

            def hT_scratch(t0, t1):
                t = apool.tile([P, (t1 - t0) * P], bf16, tag="hT2")
                hT2_tiles[(t0, t1)] = t
                return t

            layer(xg_d, xownT, wl1, wr1, bias1, True, hT_resident,
                  h_bounce, bf16)

            nc.gpsimd.collective_compute(
                "AllGather",
                mybir.AluOpType.bypass,
                replica_groups=[list(range(NCORES))],
                ins=[h_bounce[:]],
                outs=[h_full3[:]],
            )

            layer(h_full2, hT, wl2, wr2, bias2, False, hT_scratch,
                  out_d, mybir.dt.float32)

    return nc


def run(x, edge_index, W_l1, b_l1, W_r1, W_l2, b_l2, W_r2, trace=False):
    n_nodes = x.shape[0]
    assert n_nodes == NCORES * NPC

    gmaxc = int(os.environ.get("SAGE_GMAXC", "8"))  # cols per dma_gather

    ch, colof, ncols, groups, idx16, edst, erec = _prep_edges(
        np.asarray(edge_index), n_nodes
    )
    tile_cols = [
        [c for r in range(NWIN)
         for c in range(int(colof[t, r]), int(colof[t, r] + ch[t, r]))]
        for t in range(TPC)
    ]
    gcmax = max(g[3] - g[2] for g in groups)

    x = np.asarray(x, np.float32)
    x_pad = np.zeros((NALL, P), np.float32)
    for c in range(NCORES):
        x_pad[c * NPC_PAD : c * NPC_PAD + NPC] = x[c * NPC : (c + 1) * NPC]

    import ml_dtypes

    bf = ml_dtypes.bfloat16
    common = {
        "xg": x_pad.astype(bf),
        "wl1": np.asarray(W_l1, np.float32).astype(bf),
        "wr1": np.asarray(W_r1, np.float32).astype(bf),
        "wl2": np.asarray(W_l2, np.float32).astype(bf),
        "wr2": np.asarray(W_r2, np.float32).astype(bf),
        "bias1": np.asarray(b_l1, np.float32).reshape(P, 1),
        "bias2": np.asarray(b_l2, np.float32).reshape(P, 1),
        "iota": np.ascontiguousarray(
            np.broadcast_to(np.arange(P, dtype=np.float32), (P, P))
        ).astype(bf),
        "ident": np.eye(P, dtype=np.float32).astype(bf),
    }
    in_maps = []
    for c in range(NCORES):
        m = dict(common)
        m["xownT"] = np.ascontiguousarray(
            x_pad[c * NPC_PAD : (c + 1) * NPC_PAD].T
        ).astype(bf)
        m["idx16"] = idx16[c]
        m["edst"] = edst[c]
        m["erec"] = erec[c]
        in_maps.append(m)

    nc = _build_program(ncols, groups, tile_cols, gcmax, gmaxc)
    nc.finalize()

    from concourse.bass_utils import run_bass_kernel_spmd

    res = run_bass_kernel_spmd(nc, in_maps, list(range(NCORES)), trace=trace)
    out = np.empty((n_nodes, P), np.float32)
    for c in range(NCORES):
        out[c * NPC : (c + 1) * NPC] = res.results[c]["out"][:NPC]
    return out, res


def kernel(x, edge_index, W_l1, b_l1, W_r1, W_l2, b_l2, W_r2):
    out, _ = run(x, edge_index, W_l1, b_l1, W_r1, W_l2, b_l2, W_r2)
    return out


# revision 9
# speedup vs baseline: 1.2975x; 1.2590x over previous
"""Two-layer GraphSAGE (mean aggregation) on 8 Trainium2 NeuronCores.

Strategy (dst-partitioning per the hint), v3:
- Nodes partitioned by destination across 8 cores (12500 each, padded to
  12544 = 98*128 rows). Each core owns edges whose dst is in its slice,
  bucketed on host by (dst tile, src window); 4 windows of <=32768 rows
  make gather indices fit int16.
- x[src] rows are fetched in bf16 with batched SWDGE dma_gather (1024
  descriptors per instruction, rotating across the 4 SWDGE queues so
  descriptor generation pipelines with the transfers).
- Aggregation per 128-dst tile: indicator matmuls on the PE. Indicators
  (pure 0/1) are built in one DVE is_equal per (group, window) span; the
  1/deg mean scaling is applied on the PSUM drain via a host-shipped
  [128, NPC_PAD] broadcast reciprocal table.
- Downstream stays in transposed [feat, node] layout: hT = W_l^T @ aggT
  + W_r^T @ xT, so bias+relu+PSUM-drain is a single Activation op and
  layer-2's self term reuses layer-1's resident hT tile. Row-layout h
  (gather table / final output) is produced with PE transposes and one
  batched DMA per tile-group.
- Between layers the bf16 h slices are AllGathered.

kernel(**inputs) -> np.ndarray takes FULL inputs, returns FULL output.
"""

import os

import numpy as np

P = 128
NCORES = 8
NPC = 12500            # nodes per core
TPC = 98               # 128-node tiles per core
NPC_PAD = TPC * P      # 12544
NALL = NCORES * NPC_PAD  # 100352
NWIN = 4
WIN = 32768            # window size so gather indices fit int16
WLIM = [min((w + 1) * WIN, NALL) for w in range(NWIN)]
WBASE = [w * WIN for w in range(NWIN)]
TPG = 12               # tiles per group
DENSE_B = 4            # tiles per dense-matmul batch (512 moving free dim)


def _prep_edges(edge_index: np.ndarray, n_nodes: int):
    """Bucket edges by (owner core, dst tile, src window).

    Column layout (shared by all cores; chunk counts are max over cores so
    the SPMD program is uniform): group-major, then window, then tile.
    """
    src = edge_index[0].astype(np.int64)
    dst = edge_index[1].astype(np.int64)
    srcpad = (src // NPC) * NPC_PAD + (src % NPC)
    rng = srcpad >> 15  # window index, 0..3
    core = dst // NPC
    loc = dst % NPC
    tl = loc // P
    off = loc % P

    key = (core * TPC + tl) * NWIN + rng
    cnt = np.bincount(key, minlength=NCORES * TPC * NWIN).reshape(
        NCORES, TPC, NWIN
    )
    ch = -(-cnt.max(axis=0) // P)  # [TPC, NWIN], 0 allowed
    assert ch.sum(axis=1).min() >= 1

    colof = np.zeros((TPC, NWIN), np.int64)
    groups = []  # (t0, t1, gc0, gc1, spans[r] = (c0, c1))
    c = 0
    for g0 in range(0, TPC, TPG):
        t0, t1 = g0, min(g0 + TPG, TPC)
        gc0 = c
        spans = []
        for r in range(NWIN):
            rc0 = c
            for t in range(t0, t1):
                colof[t, r] = c
                c += ch[t, r]
            spans.append((rc0, c))
        groups.append((t0, t1, gc0, c, spans))
    ncols = int(c)
    S = ncols * P

    # rank of each edge within its (core, tile, window) bucket
    order = np.argsort(key, kind="stable")
    sk = key[order]
    first = np.r_[True, sk[1:] != sk[:-1]]
    idx_of_first = np.where(first)[0]
    grp_id = np.cumsum(first) - 1
    rank = np.arange(len(sk)) - idx_of_first[grp_id]
    slot = colof[tl[order], rng[order]] * P + rank

    idxflat = np.zeros((NCORES, S), np.int16)
    edstflat = np.full((NCORES, S), -1.0, np.float32)
    idxval = (srcpad - np.asarray(WBASE, np.int64)[rng]).astype(np.int16)
    co = core[order]
    idxflat[co, slot] = idxval[order]
    edstflat[co, slot] = off[order].astype(np.float32)

    deg = np.bincount(dst, minlength=n_nodes).astype(np.float64)
    recip = (1.0 / np.maximum(deg, 1.0)).astype(np.float32)  # [n_nodes]

    idx16 = np.ascontiguousarray(
        np.tile(idxflat.reshape(NCORES, S // 16, 16).transpose(0, 2, 1), (1, 8, 1))
    )
    edst = np.ascontiguousarray(
        edstflat.reshape(NCORES, ncols, P).transpose(0, 2, 1)
    )
    return ch, colof, ncols, groups, idx16, edst, recip


def _build_program(ncols, groups, tile_cols, gcmax, gmaxc):
    from concourse import bacc, library_config, mybir, tile

    f32 = mybir.dt.float32
    bf16 = mybir.dt.bfloat16
    i16 = mybir.dt.int16

    nc = bacc.Bacc(
        "TRN2",
        target_bir_lowering=False,
        debug=False,
        num_devices=NCORES,
        num_swdge_queues=4,
    )

    xg_d = nc.declare_dram_parameter("xg", [NALL, P], bf16, isOutput=False)
    xownT_d = nc.declare_dram_parameter("xownT", [P, NPC_PAD], bf16, isOutput=False)
    idx_d = nc.declare_dram_parameter("idx16", [P, (ncols * P) // 16], i16,
                                      isOutput=False)
    edst_d = nc.declare_dram_parameter("edst", [P, ncols], f32, isOutput=False)
    recipb_d = nc.declare_dram_parameter("recipb", [P, NPC_PAD], bf16,
                                         isOutput=False)
    wl1_d = nc.declare_dram_parameter("wl1", [P, P], bf16, isOutput=False)
    wr1_d = nc.declare_dram_parameter("wr1", [P, P], bf16, isOutput=False)
    wl2_d = nc.declare_dram_parameter("wl2", [P, P], bf16, isOutput=False)
    wr2_d = nc.declare_dram_parameter("wr2", [P, P], bf16, isOutput=False)
    bias1_d = nc.declare_dram_parameter("bias1", [P, 1], f32, isOutput=False)
    bias2_d = nc.declare_dram_parameter("bias2", [P, 1], f32, isOutput=False)
    iota_d = nc.declare_dram_parameter("iota", [P, P], bf16, isOutput=False)
    ident_d = nc.declare_dram_parameter("ident", [P, P], bf16, isOutput=False)
    out_d = nc.declare_dram_parameter("out", [NPC_PAD, P], f32, isOutput=True)

    is_eq = mybir.AluOpType.is_equal
    mult = mybir.AluOpType.mult

    with tile.TileContext(nc) as tc:
        with (
            tc.tile_pool(name="const", bufs=1) as cpool,
            tc.tile_pool(name="gath", bufs=2) as gpool,
            tc.tile_pool(name="indp", bufs=2) as ipool,
            tc.tile_pool(name="agg", bufs=2) as apool,
            tc.tile_pool(name="row", bufs=2) as rpool,
            tc.tile_pool(name="psacc", bufs=2, space="PSUM") as ps_a,
            tc.tile_pool(name="psh", bufs=2, space="PSUM") as ps_h,
            tc.tile_pool(name="pst", bufs=2, space="PSUM") as ps_t,
            tc.tile_pool(name="dram", bufs=1, space="DRAM") as dpool,
        ):
            # InstDMAGatherAnt lives in the mlp Q7 ucode library
            nc.gpsimd.load_library(library_config.mlp)

            def load_const(dram_ap, shape, dtype, name):
                t = cpool.tile(shape, dtype, name=name)
                nc.sync.dma_start(out=t[:], in_=dram_ap)
                return t

            wl1 = load_const(wl1_d[:], [P, P], bf16, "wl1")
            wr1 = load_const(wr1_d[:], [P, P], bf16, "wr1")
            wl2 = load_const(wl2_d[:], [P, P], bf16, "wl2")
            wr2 = load_const(wr2_d[:], [P, P], bf16, "wr2")
            bias1 = load_const(bias1_d[:], [P, 1], f32, "bias1")
            bias2 = load_const(bias2_d[:], [P, 1], f32, "bias2")
            iota = load_const(iota_d[:], [P, P], bf16, "iota")
            ident = load_const(ident_d[:], [P, P], bf16, "ident")
            xownT = load_const(xownT_d[:], [P, NPC_PAD], bf16, "xownT")
            idx16 = load_const(idx_d[:], [P, (ncols * P) // 16], i16, "idx16")
            edst = load_const(edst_d[:], [P, ncols], f32, "edst")
            recipb = load_const(recipb_d[:], [P, NPC_PAD], bf16, "recipb")

            hT = cpool.tile([P, NPC_PAD], bf16, name="hT")

            h_bounce = dpool.tile([NPC_PAD, P], bf16, name="h_bounce")
            h_full3 = dpool.tile(
                [NCORES, NPC_PAD, P], bf16, name="h_full", addr_space="Shared"
            )
            h_full2 = h_full3[:].rearrange("c n d -> (c n) d")

            qctr = [0]

            def layer(src2d, selfT, wl, wr, bias, relu, hTdst_fn, rowdst,
                      rowdtype):
                for gi, (t0, t1, gc0, gc1, spans) in enumerate(groups):
                    nt = t1 - t0
                    gbuf = gpool.tile([P, gcmax, P], bf16, tag="g")
                    ibuf = ipool.tile([P, gcmax, P], bf16, tag="i")
                    for r, (c0, c1) in enumerate(spans):
                        if c1 == c0:
                            continue
                        for s0 in range(c0, c1, gmaxc):
                            s1 = min(s0 + gmaxc, c1)
                            n_idx = (s1 - s0) * P
                            nc.gpsimd.dma_gather(
                                gbuf[:, s0 - gc0 : s1 - gc0, :],
                                src2d[WBASE[r] : WLIM[r], :],
                                idx16[:, s0 * 8 : s1 * 8],
                                n_idx,
                                n_idx,
                                P,
                                queue_num=qctr[0] % 4,
                            )
                            qctr[0] += 1
                        # 0/1 indicator for the whole span in one DVE op
                        nc.vector.tensor_tensor(
                            out=ibuf[:, c0 - gc0 : c1 - gc0, :],
                            in0=edst[:, c0:c1, None].to_broadcast(
                                [P, c1 - c0, P]
                            ),
                            in1=iota[:, None, :].to_broadcast([P, c1 - c0, P]),
                            op=is_eq,
                        )
                    agg = apool.tile([P, nt, P], bf16, tag="agg")
                    for ti, t in enumerate(range(t0, t1)):
                        cols = tile_cols[t]
                        acc = ps_a.tile([P, P], f32, tag="acc")
                        last = len(cols) - 1
                        for ci, col in enumerate(cols):
                            nc.tensor.matmul(
                                out=acc[:],
                                lhsT=gbuf[:, col - gc0, :],
                                rhs=ibuf[:, col - gc0, :],
                                start=(ci == 0),
                                stop=(ci == last),
                            )
                        # drain PSUM with the 1/deg mean scaling fused in
                        nc.vector.tensor_tensor(
                            out=agg[:, ti, :],
                            in0=acc[:],
                            in1=recipb[:, t * P : (t + 1) * P],
                            op=mult,
                        )

                    hTdst, hoff = hTdst_fn(t0, t1)
                    for b0 in range(t0, t1, DENSE_B):
                        b1 = min(b0 + DENSE_B, t1)
                        w = (b1 - b0) * P
                        hps = ps_h.tile([P, DENSE_B * P], f32, tag="h")
                        nc.tensor.matmul(
                            out=hps[:, :w], lhsT=wl[:],
                            rhs=agg[:, b0 - t0 : b1 - t0, :],
                            start=True, stop=False,
                        )
                        nc.tensor.matmul(
                            out=hps[:, :w], lhsT=wr[:],
                            rhs=selfT[:, b0 * P : b1 * P],
                            start=False, stop=True,
                        )
                        nc.scalar.activation(
                            out=hTdst[:, hoff + (b0 - t0) * P : hoff + (b1 - t0) * P],
                            in_=hps[:, :w],
                            func=(
                                mybir.ActivationFunctionType.Relu
                                if relu
                                else mybir.ActivationFunctionType.Identity
                            ),
                            bias=bias[:, 0:1],
                        )

                    rowbuf = rpool.tile([P, nt, P], rowdtype,
                                        tag=f"row{rowdtype}")
                    for ti in range(nt):
                        tps = ps_t.tile([P, P], bf16, tag="tp")
                        nc.tensor.transpose(
                            out=tps[:],
                            in_=hTdst[:, hoff + ti * P : hoff + (ti + 1) * P],
                            identity=ident[:],
                        )
                        nc.vector.tensor_copy(out=rowbuf[:, ti, :], in_=tps[:])
                    nc.sync.dma_start(
                        out=rowdst[t0 * P : t1 * P, :].rearrange(
                            "(t p) q -> p t q", p=P
                        ),
                        in_=rowbuf[:],
                    )

            def hT_resident(t0, t1):
                return hT, t0 * P

            def hT_scratch(t0, t1):
                t = apool.tile([P, (t1 - t0) * P], bf16, tag="hT2", name="hT2")
                return t, 0

            layer(xg_d, xownT, wl1, wr1, bias1, True, hT_resident,
                  h_bounce, mybir.dt.bfloat16)

            nc.gpsimd.collective_compute(
                "AllGather",
                mybir.AluOpType.bypass,
                replica_groups=[list(range(NCORES))],
                ins=[h_bounce[:]],
                outs=[h_full3[:]],
            )

            layer(h_full2, hT, wl2, wr2, bias2, False, hT_scratch,
                  out_d, mybir.dt.float32)

    return nc


def run(x, edge_index, W_l1, b_l1, W_r1, W_l2, b_l2, W_r2, trace=False):
    n_nodes = x.shape[0]
    assert n_nodes == NCORES * NPC

    gmaxc = int(os.environ.get("SAGE_GMAXC", "8"))  # cols per dma_gather

    ch, colof, ncols, groups, idx16, edst, recip = _prep_edges(
        np.asarray(edge_index), n_nodes
    )
    tile_cols = [
        [c for r in range(NWIN)
         for c in range(int(colof[t, r]), int(colof[t, r] + ch[t, r]))]
        for t in range(TPC)
    ]
    gcmax = max(g[3] - g[2] for g in groups)

    x = np.asarray(x, np.float32)
    x_pad = np.zeros((NALL, P), np.float32)
    for c in range(NCORES):
        x_pad[c * NPC_PAD : c * NPC_PAD + NPC] = x[c * NPC : (c + 1) * NPC]

    import ml_dtypes

    bf = ml_dtypes.bfloat16
    common = {
        "xg": x_pad.astype(bf),
        "wl1": np.asarray(W_l1, np.float32).astype(bf),
        "wr1": np.asarray(W_r1, np.float32).astype(bf),
        "wl2": np.asarray(W_l2, np.float32).astype(bf),
        "wr2": np.asarray(W_r2, np.float32).astype(bf),
        "bias1": np.asarray(b_l1, np.float32).reshape(P, 1),
        "bias2": np.asarray(b_l2, np.float32).reshape(P, 1),
        "iota": np.ascontiguousarray(
            np.broadcast_to(np.arange(P, dtype=np.float32), (P, P))
        ).astype(bf),
        "ident": np.eye(P, dtype=np.float32).astype(bf),
    }
    in_maps = []
    for c in range(NCORES):
        m = dict(common)
        m["xownT"] = np.ascontiguousarray(
            x_pad[c * NPC_PAD : (c + 1) * NPC_PAD].T
        ).astype(bf)
        m["idx16"] = idx16[c]
        m["edst"] = edst[c]
        rb = np.zeros(NPC_PAD, np.float32)
        rb[:NPC] = recip[c * NPC : (c + 1) * NPC]
        m["recipb"] = np.ascontiguousarray(
            np.broadcast_to(rb, (P, NPC_PAD))
        ).astype(bf)
        in_maps.append(m)

    nc = _build_program(ncols, groups, tile_cols, gcmax, gmaxc)
    nc.finalize()

    from concourse.bass_utils import run_bass_kernel_spmd

    res = run_bass_kernel_spmd(nc, in_maps, list(range(NCORES)), trace=trace)
    out = np.empty((n_nodes, P), np.float32)
    for c in range(NCORES):
        out[c * NPC : (c + 1) * NPC] = res.results[c]["out"][:NPC]
    return out, res


def kernel(x, edge_index, W_l1, b_l1, W_r1, W_l2, b_l2, W_r2):
    out, _ = run(x, edge_index, W_l1, b_l1, W_r1, W_l2, b_l2, W_r2)
    return out
